# revision 31
# baseline (speedup 1.0000x reference)
"""Izhikevich 2-layer SNN kernel for 8 Trainium2 NeuronCores — v2.

Reference computation (per timestep t of 100):
    cur1 = x_t @ W1.T + b1                 # [B, 100]
    spk1, v1, u1 = izh(cur1, v1, u1)
    cur2 = spk1 @ W2.T + b2                # [B, 10]
    spk2, v2, u2 = izh(cur2, v2, u2)
    record spk2, v2
Output: (spk2_rec, mem2_rec), each [100, B, 10].

Sharding: pure data parallel over batch (2048 -> 8 x 256), weights replicated.

v2 design (vs v1 baseline at ~412us):
  * Both layers fused in ONE [110, Bc] tile (rows 0:100 = L1, rows 100:110 =
    L2 delayed by TWO steps so the spk1 -> matmul -> PSUM -> state feedback
    spans two pipeline periods and stays off the critical recurrence).
  * Shifted state:  vt := v + 65,  mu := u + 20 - beta_eff - K with
    K = 407 - beta_eff chosen so the u-recurrence has NO constant term:
      vt+ = Square(0.2 vt + 2) + (P - K - mu)     P = raw matmul accum
      s'  = (vt+ < 65.03)          (INVERTED spike mask, 0/1)
      vt' = s' * vt+               (reset-to-c becomes multiply-by-mask)
      mu' = (1-a) mu + a b vt - d s'
      mu0 = -402 (uniform)
    The L2 matmul consumes sd = -d s' directly with stationary W2.T/d; the
    constant input part folds into beta2_eff = b2 + W2.sum(axis=1).
  * bf16 for x / weights / vt / masks -> halves DMA and unlocks DVE 2x/4x
    perf modes; f32 for the mu path (keeps per-neuron bias precision, which
    only survives in the -K bias of the PSUM copy).
  * Engine split per step:
      PE   m1..m7 (x chunks) + m8 (sd(t-2))             -> PSUM [110,Bc]
      Act  q = Square-act(vt); pc = Identity(P + (-K)) -> f32 SBUF
      Pool w = pc - mu; mud = (1-a)*mu; mu' = t1 + sd
      DVE  vn = q + w; sp; sd; reset; t1 = stt(vt, ab, mud)
  * Outputs (L2 rows of sp / vt) DMA'd per step in bf16; host converts:
    spk2 = 1 - sp, mem2 = vt - 65.

Host-side x layout: [T/TB, 112, TB*7*Bc] bf16 per core so each DMA is one
contiguous slab and the 784-contraction maps to 7 chained K=112 matmuls.
Stationaries are padded to M=110 (PE PSUM writes need base partition 0/32/64)
so all 8 matmuls accumulate the full fused [110, Bc] region at base 0.
"""

import os
from contextlib import ExitStack

import numpy as np
from ml_dtypes import bfloat16

import concourse.bass as bass
import concourse.bacc as bacc
import concourse.mybir as mybir
import concourse.tile as tile
from concourse.bass_utils import run_bass_kernel_spmd

# Izhikevich RS config + threshold (matches reference)
A_, B_, C_, D_ = 0.02, 0.2, -65.0, 8.0
THR = 0.03
THRS = THR - C_  # 65.03 threshold in shifted coords

T, F, H, O = 100, 784, 100, 10
HO = H + O  # 110 fused rows
P, KC = 112, 7  # F == KC * P
NCORES = 8
BATCH = 2048
BC = BATCH // NCORES  # 256 batch per core

TB = 2   # timesteps per x DMA slab
LAG = 2  # L2 pipeline delay in steps
MU0 = -402.0  # uniform mu init = (5 - beta) - (407 - beta)

LAST_RUN = None  # BassKernelResults of the most recent kernel() call


def build_program(nc, ctx, tc, Bc=BC, T_=T, TB_=TB):
    f32 = mybir.dt.float32
    bf16 = mybir.dt.bfloat16
    AL = mybir.AluOpType
    AF = mybir.ActivationFunctionType

    xT = nc.dram_tensor("xT", [T_ // TB_, P, TB_ * KC * Bc], bf16, kind="ExternalInput").ap()
    # w1t chunk k: cols 0:100 = W1_k, cols 100:110 = 0
    # w2t:         cols 0:100 = 0,    cols 100:110 = W2.T/d
    w1 = nc.dram_tensor("w1t", [P, KC * HO], bf16, kind="ExternalInput").ap()
    w2 = nc.dram_tensor("w2t", [H, HO], bf16, kind="ExternalInput").ap()
    mi = nc.dram_tensor("mi", [O, Bc], f32, kind="ExternalInput").ap()   # mu re-init
    vi = nc.dram_tensor("vi", [O, Bc], bf16, kind="ExternalInput").ap()  # vt re-init
    zi = nc.dram_tensor("zi", [O, Bc], bf16, kind="ExternalInput").ap()  # sd re-init
    ki = nc.dram_tensor("ki", [HO, 1], f32, kind="ExternalInput").ap()   # -K column
    out = nc.dram_tensor("out", [O, T_, Bc], bf16, kind="ExternalOutput").ap()

    const = ctx.enter_context(tc.tile_pool(name="const", bufs=1))
    state = ctx.enter_context(tc.tile_pool(name="state", bufs=1))
    xpool = ctx.enter_context(tc.tile_pool(name="x", bufs=4))
    qpool = ctx.enter_context(tc.tile_pool(name="q", bufs=2))
    ppool = ctx.enter_context(tc.tile_pool(name="pc", bufs=2))
    wpool = ctx.enter_context(tc.tile_pool(name="w", bufs=4))
    vnpool = ctx.enter_context(tc.tile_pool(name="vn", bufs=2))
    sppool = ctx.enter_context(tc.tile_pool(name="sp", bufs=3))
    sdpool = ctx.enter_context(tc.tile_pool(name="sd", bufs=4))
    mdpool = ctx.enter_context(tc.tile_pool(name="md", bufs=2))
    t1pool = ctx.enter_context(tc.tile_pool(name="t1", bufs=2))
    pp = ctx.enter_context(tc.tile_pool(name="ps", bufs=4, space="PSUM"))
    ppt = ctx.enter_context(tc.tile_pool(name="pst", bufs=2, space="PSUM"))

    w1sb = const.tile([P, KC * HO], bf16)
    nc.sync.dma_start(w1sb[:], w1)
    w2sb = const.tile([H, HO], bf16)
    nc.sync.dma_start(w2sb[:], w2)
    kisb = const.tile([HO, 1], f32)
    nc.sync.dma_start(kisb[:], ki)
    zeroHO = const.tile([HO, Bc], bf16)
    nc.vector.memset(zeroHO[:], 0.0)
    b2c = const.tile([HO, 1], f32)
    nc.vector.memset(b2c[:], 2.0)

    # persistent state: vt ping-pong x3 (bf16), mu ping-pong x2 (f32)
    vbuf = [state.tile([HO, Bc], bf16, tag=f"v{i}", name=f"v{i}") for i in range(3)]
    nc.vector.memset(vbuf[0][:], -70.0 - C_)  # vt0 = -5
    mbuf = [state.tile([HO, Bc], f32, tag=f"m{i}", name=f"m{i}") for i in range(2)]
    nc.vector.memset(mbuf[0][:], MU0)

    sds = {}  # step -> sd tile (L2 matmul at t+LAG, w2 and ybz at t+1)

    def izh_step(t, pslice, rows, sd_lag):
        """Emit one fused izhikevich update on `rows` using PSUM ap `pslice`.

        State rho(t) := mu(t) - sd(t) defers the u-jump by one step EXACTLY:
          rho(t) = 0.98 rho(t-1) + [zb(t) + 0.98 sd(t-1)]
          w(t)   = pc(t) - rho(t-1) - sd(t-1)
        This keeps the current-step spike out of the mu -> w -> spike -> mu
        distance-1 recurrence without changing the math.
        """
        vcur = vbuf[t % 3]
        vnxt = vbuf[(t + 1) % 3]
        mcur = mbuf[t % 2]
        mnxt = mbuf[(t + 1) % 2]

        # Act: pc = P + (-K) -> f32 (FIRST: its PSUM input is ready early,
        # PE runs ahead); then q = Square(0.2 vt + 2)
        pc = ppool.tile([HO, Bc], f32, tag="pc")
        nc.scalar.activation(pc[rows, :], pslice, AF.Identity,
                             bias=kisb[rows, 0:1], scale=1.0)
        q = qpool.tile([HO, Bc], bf16, tag="q")
        nc.scalar.activation(q[rows, :], vcur[rows, :], AF.Square,
                             bias=b2c[rows, 0:1], scale=0.2)

        # Pool: w = pc - rho - sd(t-1)
        w1 = wpool.tile([HO, Bc], bf16, tag="w1")
        nc.gpsimd.tensor_tensor(w1[rows, :], pc[rows, :], mcur[rows, :], AL.subtract)
        if sd_lag is None:
            w = w1
        else:
            w = wpool.tile([HO, Bc], bf16, tag="w2")
            nc.gpsimd.tensor_tensor(w[rows, :], w1[rows, :], sd_lag[rows, :],
                                    AL.subtract)

        # DVE. The rho-update (zb, ybz, stt) is emitted FIRST: it depends
        # only on state + lagged tiles, so rho(t) lands early and w1(t+1)
        # (which waits on it) is off the critical distance-1 path.
        zb = mdpool.tile([HO, Bc], bf16, tag="zb")
        nc.vector.tensor_scalar(zb[rows, :], vcur[rows, :], A_ * B_, None, AL.mult)
        ybz = t1pool.tile([HO, Bc], bf16, tag="ybz")
        if sd_lag is None:
            nc.vector.tensor_copy(ybz[rows, :], zb[rows, :])
        else:
            nc.vector.scalar_tensor_tensor(ybz[rows, :], sd_lag[rows, :], 0.98,
                                           zb[rows, :], AL.mult, AL.add)
        nc.vector.scalar_tensor_tensor(mnxt[rows, :], mcur[rows, :], 1.0 - A_,
                                       ybz[rows, :], AL.mult, AL.add)
        # v-chain
        vn = vnpool.tile([HO, Bc], bf16, tag="vn")
        nc.vector.tensor_tensor(vn[rows, :], q[rows, :], w[rows, :], AL.add)
        sp = sppool.tile([HO, Bc], bf16, tag="sp")
        nc.vector.tensor_scalar(sp[rows, :], vn[rows, :], THRS, None, AL.is_lt)
        nc.vector.tensor_tensor(vnxt[rows, :], sp[rows, :], vn[rows, :], AL.mult)
        sd = sdpool.tile([HO, Bc], bf16, tag="sd")
        nc.vector.tensor_scalar(sd[rows, :], vn[rows, :], THRS, -D_, AL.is_lt, AL.mult)

        sds[t] = sd
        return sp, sd, vnxt

    for tb in range(T_ // TB_):
        xt = xpool.tile([P, TB_ * KC * Bc], bf16)
        # alternate x slabs across the two HW DGE queues (SP / Activation)
        (nc.sync if tb % 2 == 0 else nc.scalar).dma_start(xt[:], xT[tb, :, :])
        for tt in range(TB_):
            t = tb * TB_ + tt

            p = pp.tile([HO, Bc], f32)
            for k in range(KC):
                nc.tensor.matmul(
                    p[:],
                    w1sb[:, k * HO:(k + 1) * HO],
                    xt[:, (tt * KC + k) * Bc:(tt * KC + k + 1) * Bc],
                    start=(k == 0),
                    stop=False,
                )
            nc.tensor.matmul(
                p[:], w2sb[:, :],
                zeroHO[0:H, :] if t < LAG else sds[t - LAG][0:H, :],
                start=False, stop=True,
            )
            if t - LAG - 1 in sds:
                del sds[t - LAG - 1]

            sp, sd, vnxt = izh_step(t, p[:], slice(0, HO),
                                    None if t == 0 else sds[t - 1])

            if t >= LAG:
                nc.sync.dma_start(out[:, t - LAG, :], vnxt[H:HO, :])

            if t == LAG - 1:
                # L2 rows computed garbage during warmup: re-init state via
                # DMA (compute engines need 32-aligned partition bases).
                # sd(1) L2 rows must be zeroed too: ybz(2) reads them.
                nc.sync.dma_start(vnxt[H:HO, :], vi)
                nc.sync.dma_start(mbuf[(t + 1) % 2][H:HO, :], mi)
                nc.sync.dma_start(sd[H:HO, :], zi)

    # tail: two L2-only steps (consume spk1 of t=98,99 -> outputs 98,99).
    # Compute rows 64:110 (base-64 aligned); rows 64:100 are dead L1 state,
    # and the w2sb slice supplies zero columns for them.
    for t in range(T_, T_ + LAG):
        pt = ppt.tile([HO, Bc], f32, tag="pt")
        nc.tensor.matmul(pt[64:HO, :], w2sb[:, 64:HO], sds[t - LAG][0:H, :],
                         start=True, stop=True)
        sp, sd, vnxt = izh_step(t, pt[64:HO, :], slice(64, HO), sds[t - 1])
        nc.sync.dma_start(out[:, t - LAG, :], vnxt[H:HO, :])


def _host_inputs(x, W1, b1, W2, b2, Bc=BC, T_=T, TB_=TB):
    """Per-core input dicts. x: [BATCH, T, F] fp32."""
    w1c = W1.reshape(H, KC, P).transpose(2, 1, 0)          # [P, KC, H]
    w1p = np.zeros((P, KC, HO), np.float32)
    w1p[:, :, :H] = w1c
    w1t = np.ascontiguousarray(w1p).reshape(P, KC * HO).astype(bfloat16)
    w2p = np.zeros((H, HO), np.float32)
    w2p[:, H:] = W2.T / D_
    w2t = np.ascontiguousarray(w2p).astype(bfloat16)

    beta = np.concatenate([b1, b2 + W2.sum(axis=1)]).astype(np.float64)  # [110]
    kv = np.ascontiguousarray((beta - 407.0).astype(np.float32)[:, None])  # -K

    n_cores = x.shape[0] // Bc
    in_maps = []
    for i in range(n_cores):
        xs = x[i * Bc:(i + 1) * Bc]  # [Bc, T, F]
        xTi = np.ascontiguousarray(
            xs.reshape(Bc, T_ // TB_, TB_, KC, P).transpose(1, 4, 2, 3, 0)
        ).reshape(T_ // TB_, P, TB_ * KC * Bc).astype(bfloat16)
        in_maps.append({
            "xT": xTi, "w1t": w1t, "w2t": w2t, "ki": kv,
            "mi": np.full((O, Bc), MU0, dtype=np.float32),
            "vi": np.full((O, Bc), -70.0 - C_, dtype=bfloat16),
            "zi": np.zeros((O, Bc), dtype=bfloat16),
        })
    return in_maps


def _install_ntff_shim():
    """Register the NTFF profile hook when the image's antenv lacks axon_hooks.

    Only needed for BASS_TRACE profiling runs; silently a no-op if anything
    is missing so plain correctness runs never depend on it.
    """
    import sys
    import types
    try:
        import antenv.axon_hooks  # noqa: F401  # already present: nothing to do
        return
    except ImportError:
        pass
    try:
        from trn_agent_boot.trn_boot import _ntff_profile_via_ctypes
        hook = _ntff_profile_via_ctypes("/opt/axon/libaxon_pjrt.so")
        mod = types.ModuleType("antenv.axon_hooks")
        mod._hook = hook
        mod.get_axon_ntff_profile_hook = lambda: mod._hook
        mod.set_axon_ntff_profile_hook = lambda h: setattr(mod, "_hook", h)
        sys.modules["antenv.axon_hooks"] = mod
    except Exception:
        pass


def kernel(x, W1, b1, W2, b2):
    global LAST_RUN
    if os.environ.get("BASS_TRACE"):
        _install_ntff_shim()
    x = np.ascontiguousarray(x, dtype=np.float32)
    W1 = np.asarray(W1, np.float32)
    b1 = np.asarray(b1, np.float32)
    W2 = np.asarray(W2, np.float32)
    b2 = np.asarray(b2, np.float32)

    nc = bacc.Bacc("TRN2", target_bir_lowering=False, debug=False,
                   num_devices=NCORES)
    with tile.TileContext(nc) as tc:
        with ExitStack() as ctx:
            build_program(nc, ctx, tc)
    nc.compile()

    in_maps = _host_inputs(x, W1, b1, W2, b2)
    res = run_bass_kernel_spmd(
        nc, in_maps, core_ids=list(range(NCORES)),
        trace=bool(os.environ.get("BASS_TRACE")),
    )
    LAST_RUN = res

    spk = np.empty((T, BATCH, O), np.float32)
    mem = np.empty((T, BATCH, O), np.float32)
    for i in range(NCORES):
        o = res.results[i]["out"].astype(np.float32)  # [O, T, Bc], shifted vt
        # spike <=> post-reset vt == 0 exactly (reference v2 stays in
        # [-70, -68.1], far from the reset value, so no false positives)
        spk[:, i * BC:(i + 1) * BC, :] = (o == 0.0).transpose(1, 2, 0)
        mem[:, i * BC:(i + 1) * BC, :] = (o + C_).transpose(1, 2, 0)
    return spk, mem


# revision 32
# speedup vs baseline: 1.0432x; 1.0432x over previous
"""Izhikevich 2-layer SNN kernel for 8 Trainium2 NeuronCores — v2.

Reference computation (per timestep t of 100):
    cur1 = x_t @ W1.T + b1                 # [B, 100]
    spk1, v1, u1 = izh(cur1, v1, u1)
    cur2 = spk1 @ W2.T + b2                # [B, 10]
    spk2, v2, u2 = izh(cur2, v2, u2)
    record spk2, v2
Output: (spk2_rec, mem2_rec), each [100, B, 10].

Sharding: pure data parallel over batch (2048 -> 8 x 256), weights replicated.

v2 design (vs v1 baseline at ~412us):
  * Both layers fused in ONE [110, Bc] tile (rows 0:100 = L1, rows 100:110 =
    L2 delayed by TWO steps so the spk1 -> matmul -> PSUM -> state feedback
    spans two pipeline periods and stays off the critical recurrence).
  * Shifted state:  vt := v + 65,  mu := u + 20 - beta_eff - K with
    K = 407 - beta_eff chosen so the u-recurrence has NO constant term:
      vt+ = Square(0.2 vt + 2) + (P - K - mu)     P = raw matmul accum
      s'  = (vt+ < 65.03)          (INVERTED spike mask, 0/1)
      vt' = s' * vt+               (reset-to-c becomes multiply-by-mask)
      mu' = (1-a) mu + a b vt - d s'
      mu0 = -402 (uniform)
    The L2 matmul consumes sd = -d s' directly with stationary W2.T/d; the
    constant input part folds into beta2_eff = b2 + W2.sum(axis=1).
  * bf16 for x / weights / vt / masks -> halves DMA and unlocks DVE 2x/4x
    perf modes; f32 for the mu path (keeps per-neuron bias precision, which
    only survives in the -K bias of the PSUM copy).
  * Engine split per step:
      PE   m1..m7 (x chunks) + m8 (sd(t-2))             -> PSUM [110,Bc]
      Act  q = Square-act(vt); pc = Identity(P + (-K)) -> f32 SBUF
      Pool w = pc - mu; mud = (1-a)*mu; mu' = t1 + sd
      DVE  vn = q + w; sp; sd; reset; t1 = stt(vt, ab, mud)
  * Outputs (L2 rows of sp / vt) DMA'd per step in bf16; host converts:
    spk2 = 1 - sp, mem2 = vt - 65.

Host-side x layout: [T/TB, 112, TB*7*Bc] bf16 per core so each DMA is one
contiguous slab and the 784-contraction maps to 7 chained K=112 matmuls.
Stationaries are padded to M=110 (PE PSUM writes need base partition 0/32/64)
so all 8 matmuls accumulate the full fused [110, Bc] region at base 0.
"""

import os
from contextlib import ExitStack

import numpy as np
from ml_dtypes import bfloat16

import concourse.bass as bass
import concourse.bacc as bacc
import concourse.mybir as mybir
import concourse.tile as tile
from concourse.bass_utils import run_bass_kernel_spmd

# Izhikevich RS config + threshold (matches reference)
A_, B_, C_, D_ = 0.02, 0.2, -65.0, 8.0
THR = 0.03
THRS = THR - C_  # 65.03 threshold in shifted coords

T, F, H, O = 100, 784, 100, 10
HO = H + O  # 110 fused rows
P, KC = 112, 7  # F == KC * P
NCORES = 8
BATCH = 2048
BC = BATCH // NCORES  # 256 batch per core

TB = 2   # timesteps per x DMA slab
LAG = 2  # L2 pipeline delay in steps
MU0 = -402.0  # uniform mu init = (5 - beta) - (407 - beta)

LAST_RUN = None  # BassKernelResults of the most recent kernel() call


def build_program(nc, ctx, tc, Bc=BC, T_=T, TB_=TB):
    f32 = mybir.dt.float32
    bf16 = mybir.dt.bfloat16
    AL = mybir.AluOpType
    AF = mybir.ActivationFunctionType

    xT = nc.dram_tensor("xT", [T_ // TB_, P, TB_ * KC * Bc], bf16, kind="ExternalInput").ap()
    # w1t chunk k: cols 0:100 = W1_k, cols 100:110 = 0
    # w2t:         cols 0:100 = 0,    cols 100:110 = W2.T/d
    w1 = nc.dram_tensor("w1t", [P, KC * HO], bf16, kind="ExternalInput").ap()
    w2 = nc.dram_tensor("w2t", [H, HO], bf16, kind="ExternalInput").ap()
    mi = nc.dram_tensor("mi", [O, Bc], f32, kind="ExternalInput").ap()   # mu re-init
    vi = nc.dram_tensor("vi", [O, Bc], bf16, kind="ExternalInput").ap()  # vt re-init
    zi = nc.dram_tensor("zi", [O, Bc], bf16, kind="ExternalInput").ap()  # sd re-init
    ki = nc.dram_tensor("ki", [HO, 1], f32, kind="ExternalInput").ap()   # -K column
    out = nc.dram_tensor("out", [O, T_, Bc], bf16, kind="ExternalOutput").ap()

    const = ctx.enter_context(tc.tile_pool(name="const", bufs=1))
    state = ctx.enter_context(tc.tile_pool(name="state", bufs=1))
    xpool = ctx.enter_context(tc.tile_pool(name="x", bufs=4))
    qpool = ctx.enter_context(tc.tile_pool(name="q", bufs=2))
    ppool = ctx.enter_context(tc.tile_pool(name="pc", bufs=2))
    wpool = ctx.enter_context(tc.tile_pool(name="w", bufs=4))
    vnpool = ctx.enter_context(tc.tile_pool(name="vn", bufs=2))
    sppool = ctx.enter_context(tc.tile_pool(name="sp", bufs=3))
    sd98pool = ctx.enter_context(tc.tile_pool(name="sd98", bufs=3))
    sdpool = ctx.enter_context(tc.tile_pool(name="sd", bufs=4))
    mdpool = ctx.enter_context(tc.tile_pool(name="md", bufs=2))
    t1pool = ctx.enter_context(tc.tile_pool(name="t1", bufs=2))
    pp = ctx.enter_context(tc.tile_pool(name="ps", bufs=4, space="PSUM"))
    ppt = ctx.enter_context(tc.tile_pool(name="pst", bufs=2, space="PSUM"))

    w1sb = const.tile([P, KC * HO], bf16)
    nc.sync.dma_start(w1sb[:], w1)
    w2sb = const.tile([H, HO], bf16)
    nc.sync.dma_start(w2sb[:], w2)
    kisb = const.tile([HO, 1], f32)
    nc.sync.dma_start(kisb[:], ki)
    zeroHO = const.tile([HO, Bc], bf16)
    nc.vector.memset(zeroHO[:], 0.0)
    b2c = const.tile([HO, 1], f32)
    nc.vector.memset(b2c[:], 2.0)

    # persistent state: vt ping-pong x3 (bf16), mu ping-pong x2 (f32)
    vbuf = [state.tile([HO, Bc], bf16, tag=f"v{i}", name=f"v{i}") for i in range(3)]
    nc.vector.memset(vbuf[0][:], -70.0 - C_)  # vt0 = -5
    mbuf = [state.tile([HO, Bc], f32, tag=f"m{i}", name=f"m{i}") for i in range(2)]
    nc.vector.memset(mbuf[0][:], MU0)

    sds = {}    # step -> sd tile (L2 matmul at t+LAG, w2 at t+1)
    sd98s = {}  # step -> sd98 tile (ybz at t+1)

    def izh_step(t, pslice, rows, sd_lag, sd98_lag):
        """Emit one fused izhikevich update on `rows` using PSUM ap `pslice`.

        State rho(t) := mu(t) - sd(t) defers the u-jump by one step EXACTLY:
          rho(t) = 0.98 rho(t-1) + [zb(t) + 0.98 sd(t-1)]
          w(t)   = pc(t) - rho(t-1) - sd(t-1)
        This keeps the current-step spike out of the mu -> w -> spike -> mu
        distance-1 recurrence without changing the math.
        """
        vcur = vbuf[t % 3]
        vnxt = vbuf[(t + 1) % 3]
        mcur = mbuf[t % 2]
        mnxt = mbuf[(t + 1) % 2]

        # Act: pc = P + (-K) -> f32 (FIRST: its PSUM input is ready early,
        # PE runs ahead); then q = Square(0.2 vt + 2)
        pc = ppool.tile([HO, Bc], f32, tag="pc")
        nc.scalar.activation(pc[rows, :], pslice, AF.Identity,
                             bias=kisb[rows, 0:1], scale=1.0)
        q = qpool.tile([HO, Bc], bf16, tag="q")
        nc.scalar.activation(q[rows, :], vcur[rows, :], AF.Square,
                             bias=b2c[rows, 0:1], scale=0.2)

        # Pool: w = pc - rho - sd(t-1)
        w1 = wpool.tile([HO, Bc], bf16, tag="w1")
        nc.gpsimd.tensor_tensor(w1[rows, :], pc[rows, :], mcur[rows, :], AL.subtract)
        if sd_lag is None:
            w = w1
        else:
            w = wpool.tile([HO, Bc], bf16, tag="w2")
            nc.gpsimd.tensor_tensor(w[rows, :], w1[rows, :], sd_lag[rows, :],
                                    AL.subtract)

        # DVE. The rho-update (zb, ybz, stt) is emitted FIRST: it depends
        # only on state + lagged tiles, so rho(t) lands early and w1(t+1)
        # (which waits on it) is off the critical distance-1 path.
        zb = mdpool.tile([HO, Bc], bf16, tag="zb")
        nc.vector.tensor_scalar(zb[rows, :], vcur[rows, :], A_ * B_, None, AL.mult)
        ybz = t1pool.tile([HO, Bc], bf16, tag="ybz")
        if sd98_lag is None:
            nc.vector.tensor_copy(ybz[rows, :], zb[rows, :])
        else:
            nc.vector.tensor_tensor(ybz[rows, :], zb[rows, :], sd98_lag[rows, :],
                                    AL.add)
        nc.vector.scalar_tensor_tensor(mnxt[rows, :], mcur[rows, :], 1.0 - A_,
                                       ybz[rows, :], AL.mult, AL.add)
        # v-chain
        vn = vnpool.tile([HO, Bc], bf16, tag="vn")
        nc.vector.tensor_tensor(vn[rows, :], q[rows, :], w[rows, :], AL.add)
        sp = sppool.tile([HO, Bc], bf16, tag="sp")
        nc.vector.tensor_scalar(sp[rows, :], vn[rows, :], THRS, None, AL.is_lt)
        nc.vector.tensor_tensor(vnxt[rows, :], sp[rows, :], vn[rows, :], AL.mult)
        sd = sdpool.tile([HO, Bc], bf16, tag="sd")
        nc.vector.tensor_scalar(sd[rows, :], vn[rows, :], THRS, -D_, AL.is_lt, AL.mult)
        sd98 = sd98pool.tile([HO, Bc], bf16, tag="sd98")
        nc.vector.tensor_scalar(sd98[rows, :], vn[rows, :], THRS, -0.98 * D_,
                                AL.is_lt, AL.mult)

        sds[t] = sd
        sd98s[t] = sd98
        return sp, sd, vnxt

    for tb in range(T_ // TB_):
        xt = xpool.tile([P, TB_ * KC * Bc], bf16)
        # alternate x slabs across the two HW DGE queues (SP / Activation)
        (nc.sync if tb % 2 == 0 else nc.scalar).dma_start(xt[:], xT[tb, :, :])
        for tt in range(TB_):
            t = tb * TB_ + tt

            p = pp.tile([HO, Bc], f32)
            for k in range(KC):
                nc.tensor.matmul(
                    p[:],
                    w1sb[:, k * HO:(k + 1) * HO],
                    xt[:, (tt * KC + k) * Bc:(tt * KC + k + 1) * Bc],
                    start=(k == 0),
                    stop=False,
                )
            nc.tensor.matmul(
                p[:], w2sb[:, :],
                zeroHO[0:H, :] if t < LAG else sds[t - LAG][0:H, :],
                start=False, stop=True,
            )
            if t - LAG - 1 in sds:
                del sds[t - LAG - 1]
            if t - 2 in sd98s:
                del sd98s[t - 2]

            sp, sd, vnxt = izh_step(t, p[:], slice(0, HO),
                                    None if t == 0 else sds[t - 1],
                                    None if t == 0 else sd98s[t - 1])

            if t >= LAG:
                nc.sync.dma_start(out[:, t - LAG, :], vnxt[H:HO, :])

            if t == LAG - 1:
                # L2 rows computed garbage during warmup: re-init state via
                # DMA (compute engines need 32-aligned partition bases).
                # sd(1) L2 rows must be zeroed too: ybz(2) reads them.
                nc.sync.dma_start(vnxt[H:HO, :], vi)
                nc.sync.dma_start(mbuf[(t + 1) % 2][H:HO, :], mi)
                nc.sync.dma_start(sd[H:HO, :], zi)
                nc.sync.dma_start(sd98s[t][H:HO, :], zi)

    # tail: two L2-only steps (consume spk1 of t=98,99 -> outputs 98,99).
    # Compute rows 64:110 (base-64 aligned); rows 64:100 are dead L1 state,
    # and the w2sb slice supplies zero columns for them.
    for t in range(T_, T_ + LAG):
        pt = ppt.tile([HO, Bc], f32, tag="pt")
        nc.tensor.matmul(pt[64:HO, :], w2sb[:, 64:HO], sds[t - LAG][0:H, :],
                         start=True, stop=True)
        sp, sd, vnxt = izh_step(t, pt[64:HO, :], slice(64, HO), sds[t - 1], sd98s[t - 1])
        nc.sync.dma_start(out[:, t - LAG, :], vnxt[H:HO, :])


def _host_inputs(x, W1, b1, W2, b2, Bc=BC, T_=T, TB_=TB):
    """Per-core input dicts. x: [BATCH, T, F] fp32."""
    w1c = W1.reshape(H, KC, P).transpose(2, 1, 0)          # [P, KC, H]
    w1p = np.zeros((P, KC, HO), np.float32)
    w1p[:, :, :H] = w1c
    w1t = np.ascontiguousarray(w1p).reshape(P, KC * HO).astype(bfloat16)
    w2p = np.zeros((H, HO), np.float32)
    w2p[:, H:] = W2.T / D_
    w2t = np.ascontiguousarray(w2p).astype(bfloat16)

    beta = np.concatenate([b1, b2 + W2.sum(axis=1)]).astype(np.float64)  # [110]
    kv = np.ascontiguousarray((beta - 407.0).astype(np.float32)[:, None])  # -K

    n_cores = x.shape[0] // Bc
    in_maps = []
    for i in range(n_cores):
        xs = x[i * Bc:(i + 1) * Bc]  # [Bc, T, F]
        xTi = np.ascontiguousarray(
            xs.reshape(Bc, T_ // TB_, TB_, KC, P).transpose(1, 4, 2, 3, 0)
        ).reshape(T_ // TB_, P, TB_ * KC * Bc).astype(bfloat16)
        in_maps.append({
            "xT": xTi, "w1t": w1t, "w2t": w2t, "ki": kv,
            "mi": np.full((O, Bc), MU0, dtype=np.float32),
            "vi": np.full((O, Bc), -70.0 - C_, dtype=bfloat16),
            "zi": np.zeros((O, Bc), dtype=bfloat16),
        })
    return in_maps


def _install_ntff_shim():
    """Register the NTFF profile hook when the image's antenv lacks axon_hooks.

    Only needed for BASS_TRACE profiling runs; silently a no-op if anything
    is missing so plain correctness runs never depend on it.
    """
    import sys
    import types
    try:
        import antenv.axon_hooks  # noqa: F401  # already present: nothing to do
        return
    except ImportError:
        pass
    try:
        from trn_agent_boot.trn_boot import _ntff_profile_via_ctypes
        hook = _ntff_profile_via_ctypes("/opt/axon/libaxon_pjrt.so")
        mod = types.ModuleType("antenv.axon_hooks")
        mod._hook = hook
        mod.get_axon_ntff_profile_hook = lambda: mod._hook
        mod.set_axon_ntff_profile_hook = lambda h: setattr(mod, "_hook", h)
        sys.modules["antenv.axon_hooks"] = mod
    except Exception:
        pass


def kernel(x, W1, b1, W2, b2):
    global LAST_RUN
    if os.environ.get("BASS_TRACE"):
        _install_ntff_shim()
    x = np.ascontiguousarray(x, dtype=np.float32)
    W1 = np.asarray(W1, np.float32)
    b1 = np.asarray(b1, np.float32)
    W2 = np.asarray(W2, np.float32)
    b2 = np.asarray(b2, np.float32)

    nc = bacc.Bacc("TRN2", target_bir_lowering=False, debug=False,
                   num_devices=NCORES)
    with tile.TileContext(nc) as tc:
        with ExitStack() as ctx:
            build_program(nc, ctx, tc)
    nc.compile()

    in_maps = _host_inputs(x, W1, b1, W2, b2)
    res = run_bass_kernel_spmd(
        nc, in_maps, core_ids=list(range(NCORES)),
        trace=bool(os.environ.get("BASS_TRACE")),
    )
    LAST_RUN = res

    spk = np.empty((T, BATCH, O), np.float32)
    mem = np.empty((T, BATCH, O), np.float32)
    for i in range(NCORES):
        o = res.results[i]["out"].astype(np.float32)  # [O, T, Bc], shifted vt
        # spike <=> post-reset vt == 0 exactly (reference v2 stays in
        # [-70, -68.1], far from the reset value, so no false positives)
        spk[:, i * BC:(i + 1) * BC, :] = (o == 0.0).transpose(1, 2, 0)
        mem[:, i * BC:(i + 1) * BC, :] = (o + C_).transpose(1, 2, 0)
    return spk, mem
